# revision 14
# baseline (speedup 1.0000x reference)
"""MoE routing kernel for Trainium2 (8 NeuronCores, expert-parallel).

Problem (hardcoded shapes): B=4, S=2048, H=1024, I=4096, E=8, capacity=1024.

Mathematical simplification of the reference: softmax routing weights are
strictly positive, so the routing mask is all-ones and the stable argsort of
the (constant) mask is the identity permutation.  Consequently every expert
processes exactly tokens 0..1023 of the flattened [8192, 1024] input, and the
output is nonzero only for those tokens:

    out[n] = sum_e softmax(x[n] @ Wr.T + b)[e] * (relu(x[n] @ Wi[e]) @ Wo[e])

Sharding: expert-parallel.  Each of the 8 cores receives the same 1024-token
slice (pre-transposed to X^T on host) and the weights of ONE expert; it
computes that expert's weighted output transposed, [1024 H, 1024 tok].  The
host sums the 8 partial outputs (the MoE combine), transposes once, and
scatters into the full [4, 2048, 1024] zero tensor.

Per-core device computation (v4, all-bf16 data path):
  router:   logits^T[E, tok] = Wr_perm X^T (PE, bf16); exp with the bias
            folded into the ACT instruction; sum over the partition dim via
            ones-matmul; the softmax stages are interleaved into early
            layer-1 iterations so the PE never stalls on the DVE/ACT chain.
  layer 1:  inter^T[I, tok] = relu(Wi^T X^T)   (bf16 matmuls, bf16 store)
  layer 2:  outT[H, tok] = Wo^T inter^T        (bf16 matmuls),
            routing-weight scale fused into the PSUM->SBUF output copy.

DMA plan: the scalar (Activation) HWDGE queue carries the startup set
(router weights first, then the 8 X^T k-chunks) so the PE can start within
~1 us; the sync (SP) queue streams the 32 Wi tiles through a 4-deep pool and
prefetches all 8 Wo tiles during layer 1.  One shared 4-slot PSUM pool
(8 banks) lets the router-logit chain plus two layer-1 chains stay open
while X^T streams in.  The last output tile is processed in column halves so
the final scale-multiply + DMA pipeline, shortening the tail.
"""

import numpy as np

_CACHE = {}

B, S, H, I, E = 4, 2048, 1024, 4096, 8
CAP = 1024  # capacity = ceil(B*S/E)
N_CORES = 8
KT = H // 128   # 8 k-tiles (H on partitions)
IT = I // 128   # 32 I-tiles
HT = H // 128   # 8 output H-tiles

HALVES = ((0, 512), (512, 1024))


def _build(reps=1):
    import concourse.bacc as bacc
    import concourse.mybir as mybir
    import concourse.tile as tile

    f32 = mybir.dt.float32
    f32r = mybir.dt.float32r
    bf16 = mybir.dt.bfloat16
    AF = mybir.ActivationFunctionType

    nc = bacc.Bacc("TRN2", target_bir_lowering=False, debug=False)

    # X^T k-chunks with the (permuted) router weight columns for that k-tile
    # packed at columns CAP..CAP+E: the first DMA dispatch then carries both
    # the first X^T chunk and its router weights (HWDGE dispatches serialize
    # at ~0.6 us each, so every separate startup DMA delays the first matmul)
    xt_d = nc.dram_tensor("xt", [128, KT, CAP + E], bf16, kind="ExternalInput")
    rb_d = nc.dram_tensor("rb", [E, 1], f32, kind="ExternalInput")
    wi_d = nc.dram_tensor("wi", [IT, 128, KT, 128], bf16, kind="ExternalInput")
    wo_d = nc.dram_tensor("wo", [HT, 128, IT, 128], bf16, kind="ExternalInput")
    outT_d = nc.dram_tensor("outT", [H, CAP], f32, kind="ExternalOutput")

    with tile.TileContext(nc) as tc:
        with (
            tc.tile_pool(name="const", bufs=1) as const_pool,
            tc.tile_pool(name="wi", bufs=4) as wi_pool,
            tc.tile_pool(name="wo", bufs=8) as wo_pool,
            tc.tile_pool(name="inter", bufs=1) as inter_pool,
            tc.tile_pool(name="outs", bufs=2) as outs_pool,
            tc.tile_pool(name="small", bufs=2) as small_pool,
            tc.tile_pool(name="ps", bufs=4, space="PSUM") as ps,
        ):
            # ---- startup DMAs ----
            # one DMA per k-chunk (X^T + router weight columns).  Chunk 0
            # goes on the sync queue AHEAD of the wi stream so it is the
            # very first HWDGE dispatch (dispatches round-robin across the
            # two queues and serialize at ~0.6 us each); chunks 1..7 ride
            # the scalar queue.  The bias is dispatched last — it is only
            # needed once the exp fires (~9 us).
            xt_sb = const_pool.tile([128, KT, CAP + E], bf16)
            nc.sync.dma_start(xt_sb[:, 0, :], xt_d.ap()[:, 0, :])
            for k in range(1, KT):
                nc.scalar.dma_start(xt_sb[:, k, :], xt_d.ap()[:, k, :])
            b_sb = const_pool.tile([E, 1], f32)
            nc.scalar.dma_start(b_sb[:], rb_d.ap())

            onesf = const_pool.tile([1, 512], f32)
            nc.vector.memset(onesf[:], 1.0)
            ones_row = const_pool.tile([1, 512], f32r)
            nc.vector.tensor_copy(ones_row[:], onesf[:])
            ones8f = const_pool.tile([8, 1], f32)
            nc.vector.memset(ones8f[:], 1.0)
            ones8 = const_pool.tile([8, 1], f32r)
            nc.vector.tensor_copy(ones8[:], ones8f[:])

            inter_init = inter_pool.tile([128, IT, CAP], bf16, name="inter")

            def emit_body():
                inter = inter_init

                # -- router stage 1 + layer-1 it=0..2, interleaved per k --
                # The PE sequencer is in-order, so the emit order must give
                # it ready work while the X^T chunks stream in: after chunk
                # k lands, the router chain and three layer-1 chains each
                # advance one k-step (~1.7 us of PE work per 0.9 us chunk).
                NPRE = 3
                lt = ps.tile([128, CAP], f32, tag="big", name="lt")
                wi_ts = {}

                def load_wi(it):
                    wi_ts[it] = wi_pool.tile(
                        [128, KT, 128], bf16, name=f"wi_{it}", tag="wi"
                    )
                    nc.sync.dma_start(wi_ts[it][:], wi_d.ap()[it])

                pre_p1 = {}
                for it in range(NPRE):
                    load_wi(it)
                    pre_p1[it] = ps.tile([128, CAP], f32, tag="big", name="p1")
                for k in range(KT):
                    for lo, hi in HALVES:
                        nc.tensor.matmul(
                            lt[0:E, lo:hi],
                            xt_sb[:, k, CAP : CAP + E],
                            xt_sb[:, k, lo:hi],
                            start=(k == 0),
                            stop=(k == KT - 1),
                        )
                    for it in range(NPRE):
                        for lo, hi in HALVES:
                            nc.tensor.matmul(
                                pre_p1[it][:, lo:hi],
                                wi_ts[it][:, k, :],
                                xt_sb[:, k, lo:hi],
                                start=(k == 0),
                                stop=(k == KT - 1),
                            )
                # exp on ACT, bias folded in (max-subtraction skipped:
                # |logits| <~ 8 so exp stays in f32 range comfortably)
                ex_sb = small_pool.tile([8, CAP], f32r, name="ex")
                for lo, hi in HALVES:
                    nc.scalar.activation(
                        ex_sb[:, lo:hi], lt[0:E, lo:hi], AF.Exp, bias=b_sb[:]
                    )
                for it in range(NPRE):
                    nc.scalar.activation(
                        inter[:, it, :], pre_p1[it][:], AF.Relu
                    )

                def emit_router_sum():
                    # -- router stage 2: sum over experts + reciprocal --
                    sm = ps.tile([128, CAP], f32, tag="big", name="sm")
                    for lo, hi in HALVES:
                        nc.tensor.matmul(
                            sm[0:1, lo:hi], ones8[:], ex_sb[:, lo:hi]
                        )
                    rc = small_pool.tile([1, CAP], f32, name="rc")
                    nc.vector.reciprocal(rc[:], sm[0:1, :])
                    w_row = small_pool.tile([1, CAP], f32r, name="w_row")
                    nc.vector.tensor_mul(w_row[:], ex_sb[0:1, :], rc[:])
                    return w_row

                def emit_router_bcast(w_row):
                    # -- router stage 3: broadcast w to 128 partitions --
                    wb = ps.tile([128, CAP], f32, tag="big", name="wb")
                    for lo, hi in HALVES:
                        nc.tensor.matmul(
                            wb[:, lo:hi], ones_row[:, 0:128], w_row[:, lo:hi]
                        )
                    wb_sb = const_pool.tile([128, CAP], f32, name="wb_sb")
                    nc.vector.tensor_copy(wb_sb[:], wb[:])
                    return wb_sb

                # all 8 layer-2 weight slabs prefetch on the sync queue,
                # spread through layer 1 (the queue is otherwise streaming
                # 256 KB wi tiles with 4x slack vs PE consumption)
                wo_tiles = {}

                def prefetch_wo(ht):
                    wo_tiles[ht] = wo_pool.tile(
                        [128, IT, 128], bf16, name=f"wo_{ht}", tag="wo"
                    )
                    nc.sync.dma_start(wo_tiles[ht][:], wo_d.ap()[ht])

                # -- layer 1 (router stages 2/3 + wo prefetch interleaved) --
                w_row = None
                wb_sb = None
                for it in range(NPRE, IT):
                    load_wi(it)
                    wi_t = wi_ts.pop(it)
                    p1 = ps.tile([128, CAP], f32, tag="big", name="p1")
                    for k in range(KT):
                        for lo, hi in HALVES:
                            nc.tensor.matmul(
                                p1[:, lo:hi],
                                wi_t[:, k, :],
                                xt_sb[:, k, lo:hi],
                                start=(k == 0),
                                stop=(k == KT - 1),
                            )
                    nc.scalar.activation(inter[:, it, :], p1[:], AF.Relu)
                    if it == NPRE + 1:
                        w_row = emit_router_sum()
                    elif it == NPRE + 3:
                        wb_sb = emit_router_bcast(w_row)
                    if it >= 8 and it <= 29 and (it - 8) % 3 == 0:
                        prefetch_wo((it - 8) // 3)

                # -- layer 2: outT = Wo^T inter^T, scale fused in copy --
                for ht in range(HT):
                    wo_t = wo_tiles.pop(ht)
                    p2 = ps.tile([128, CAP], f32, tag="big", name="p2")
                    if ht < HT - 1:
                        # halves inner: each ldweights feeds two matmuls
                        for it2 in range(IT):
                            for lo, hi in HALVES:
                                nc.tensor.matmul(
                                    p2[:, lo:hi],
                                    wo_t[:, it2, :],
                                    inter[:, it2, lo:hi],
                                    start=(it2 == 0),
                                    stop=(it2 == IT - 1),
                                )
                        o = outs_pool.tile([128, CAP], f32, name="o")
                        nc.vector.tensor_mul(o[:], p2[:], wb_sb[:])
                        nc.sync.dma_start(
                            outT_d.ap()[ht * 128 : (ht + 1) * 128, :], o[:]
                        )
                    else:
                        # last tile: column halves (PSUM-bank aligned),
                        # outer, so the scale + DMA of the first half
                        # overlaps the second half's matmul chain
                        for lo, hi in HALVES:
                            for it2 in range(IT):
                                nc.tensor.matmul(
                                    p2[:, lo:hi],
                                    wo_t[:, it2, :],
                                    inter[:, it2, lo:hi],
                                    start=(it2 == 0),
                                    stop=(it2 == IT - 1),
                                )
                            o = outs_pool.tile([128, 512], f32, name="o")
                            nc.vector.tensor_mul(
                                o[:], p2[:, lo:hi], wb_sb[:, lo:hi]
                            )
                            nc.sync.dma_start(
                                outT_d.ap()[ht * 128 : (ht + 1) * 128, lo:hi],
                                o[:],
                            )

            for _rep in range(reps):
                emit_body()

    nc.compile()
    return nc


def get_nc():
    if "nc" not in _CACHE:
        _CACHE["nc"] = _build()
    return _CACHE["nc"]


def make_in_maps(x, router_w, router_b, experts_inter, experts_out):
    import ml_dtypes

    bf = ml_dtypes.bfloat16
    x_flat = np.asarray(x, dtype=np.float32).reshape(-1, H)
    xt = np.ascontiguousarray(x_flat[:CAP].T)  # [H, CAP]
    # pack to [128, KT, CAP]: xt_p[p, k, n] = xt[k*128 + p, n]
    xt_p = np.ascontiguousarray(
        xt.reshape(KT, 128, CAP).transpose(1, 0, 2)
    ).astype(bf)

    wi_bf = np.asarray(experts_inter, dtype=np.float32).astype(bf)  # [E, H, I]
    wo_bf = np.asarray(experts_out, dtype=np.float32).astype(bf)    # [E, I, H]

    in_maps = []
    for e in range(N_CORES):
        perm = [e] + [j for j in range(E) if j != e]
        rw = np.asarray(router_w, dtype=np.float32)[perm]  # [E, H]
        rb = np.asarray(router_b, dtype=np.float32)[perm]  # [E]

        # xtr[p, k, 0:CAP] = X^T chunk; xtr[p, k, CAP+e] = rw[e, k*128+p]
        xtr = np.empty((128, KT, CAP + E), dtype=bf)
        xtr[:, :, :CAP] = xt_p
        xtr[:, :, CAP:] = (
            rw.T.reshape(KT, 128, E).transpose(1, 0, 2).astype(bf)
        )

        # wi_p[it, p, k, i] = wi[k*128+p, it*128+i]
        wi_p = np.ascontiguousarray(
            wi_bf[e].reshape(KT, 128, IT, 128).transpose(2, 1, 0, 3)
        )
        # wo_p[ht, p, it, h] = wo[it*128+p, ht*128+h]
        wo_p = np.ascontiguousarray(
            wo_bf[e].reshape(IT, 128, HT, 128).transpose(2, 1, 0, 3)
        )
        in_maps.append({
            "xt": xtr,
            "rb": np.ascontiguousarray(rb[:, None]),
            "wi": wi_p,
            "wo": wo_p,
        })
    return in_maps


def combine(results):
    partial = np.zeros((H, CAP), dtype=np.float32)
    for r in results:
        partial += r["outT"]
    out = np.zeros((B * S, H), dtype=np.float32)
    out[:CAP] = partial.T
    return out.reshape(B, S, H)


def kernel(x, router_w, router_b, experts_inter, experts_out):
    from concourse import bass_utils

    nc = get_nc()
    in_maps = make_in_maps(x, router_w, router_b, experts_inter, experts_out)
    res = bass_utils.run_bass_kernel_spmd(nc, in_maps, core_ids=list(range(N_CORES)))
    return combine(res.results)


# revision 16
# speedup vs baseline: 4.9516x; 4.9516x over previous
"""MoE routing kernel for Trainium2 (8 NeuronCores, expert-parallel).

Problem (hardcoded shapes): B=4, S=2048, H=1024, I=4096, E=8, capacity=1024.

Mathematical simplification of the reference: softmax routing weights are
strictly positive, so the routing mask is all-ones and the stable argsort of
the (constant) mask is the identity permutation.  Consequently every expert
processes exactly tokens 0..1023 of the flattened [8192, 1024] input, and the
output is nonzero only for those tokens:

    out[n] = sum_e softmax(x[n] @ Wr.T + b)[e] * (relu(x[n] @ Wi[e]) @ Wo[e])

Sharding: expert-parallel.  Each of the 8 cores receives the same 1024-token
slice (pre-transposed to X^T on host) and the weights of ONE expert; it
computes that expert's weighted output transposed, [1024 H, 1024 tok].  The
host sums the 8 partial outputs (the MoE combine), transposes once, and
scatters into the full [4, 2048, 1024] zero tensor.

Per-core device computation (v4, all-bf16 data path):
  router:   logits^T[E, tok] = Wr_perm X^T (PE, bf16); exp with the bias
            folded into the ACT instruction; sum over the partition dim via
            ones-matmul; the softmax stages are interleaved into early
            layer-1 iterations so the PE never stalls on the DVE/ACT chain.
  layer 1:  inter^T[I, tok] = relu(Wi^T X^T)   (bf16 matmuls, bf16 store)
  layer 2:  outT[H, tok] = Wo^T inter^T        (bf16 matmuls),
            routing-weight scale fused into the PSUM->SBUF output copy.

DMA plan: HWDGE dispatches round-robin across the two queues and serialize
at ~0.6 us each, so the startup is laid out to get X^T flowing immediately:
the router weights ride packed inside the X^T k-chunks (columns CAP..CAP+7),
chunk 0 is the sync queue's first entry, chunks 1-7 go on the scalar queue
(whose FIFO order the scheduler preserves), and the 32 Wi tiles stream on
the sync queue through a 4-deep pool.  All 8 Wo slabs prefetch on the
scalar queue during layer 1, safely behind the startup chunks.  One shared
4-slot PSUM pool (8 banks) lets the router-logit chain plus three layer-1
chains stay open while X^T streams in; the emit order advances all four
chains one k-step per arriving chunk so the in-order PE sequencer always
has ready work.  The last output tile is processed in column halves
(PSUM-bank aligned) so the final scale-multiply + DMA pipelines into the
second half's matmul chain, shortening the tail.
"""

import numpy as np

_CACHE = {}

B, S, H, I, E = 4, 2048, 1024, 4096, 8
CAP = 1024  # capacity = ceil(B*S/E)
N_CORES = 8
KT = H // 128   # 8 k-tiles (H on partitions)
IT = I // 128   # 32 I-tiles
HT = H // 128   # 8 output H-tiles

HALVES = ((0, 512), (512, 1024))


def _build(reps=1):
    import concourse.bacc as bacc
    import concourse.mybir as mybir
    import concourse.tile as tile

    f32 = mybir.dt.float32
    f32r = mybir.dt.float32r
    bf16 = mybir.dt.bfloat16
    AF = mybir.ActivationFunctionType

    nc = bacc.Bacc("TRN2", target_bir_lowering=False, debug=False)

    # X^T k-chunks with the (permuted) router weight columns for that k-tile
    # packed at columns CAP..CAP+E: the first DMA dispatch then carries both
    # the first X^T chunk and its router weights (HWDGE dispatches serialize
    # at ~0.6 us each, so every separate startup DMA delays the first matmul)
    xt_d = nc.dram_tensor("xt", [128, KT, CAP + E], bf16, kind="ExternalInput")
    rb_d = nc.dram_tensor("rb", [E, 1], f32, kind="ExternalInput")
    wi_d = nc.dram_tensor("wi", [IT, 128, KT, 128], bf16, kind="ExternalInput")
    wo_d = nc.dram_tensor("wo", [HT, 128, IT, 128], bf16, kind="ExternalInput")
    outT_d = nc.dram_tensor("outT", [H, CAP], f32, kind="ExternalOutput")

    with tile.TileContext(nc) as tc:
        with (
            tc.tile_pool(name="const", bufs=1) as const_pool,
            tc.tile_pool(name="wi", bufs=4) as wi_pool,
            tc.tile_pool(name="wo", bufs=8) as wo_pool,
            tc.tile_pool(name="inter", bufs=1) as inter_pool,
            tc.tile_pool(name="outs", bufs=2) as outs_pool,
            tc.tile_pool(name="small", bufs=2) as small_pool,
            tc.tile_pool(name="ps", bufs=4, space="PSUM") as ps,
        ):
            # ---- startup DMAs ----
            # one DMA per k-chunk (X^T + router weight columns).  Chunk 0
            # goes on the sync queue AHEAD of the wi stream so it is the
            # very first HWDGE dispatch (dispatches round-robin across the
            # two queues and serialize at ~0.6 us each); chunks 1..7 ride
            # the scalar queue.  The bias is dispatched last — it is only
            # needed once the exp fires (~9 us).
            xt_sb = const_pool.tile([128, KT, CAP + E], bf16)
            nc.sync.dma_start(xt_sb[:, 0, :], xt_d.ap()[:, 0, :])
            for k in range(1, KT):
                nc.scalar.dma_start(xt_sb[:, k, :], xt_d.ap()[:, k, :])
            b_sb = const_pool.tile([E, 1], f32)
            nc.scalar.dma_start(b_sb[:], rb_d.ap())

            onesf = const_pool.tile([1, 512], f32)
            nc.vector.memset(onesf[:], 1.0)
            ones_row = const_pool.tile([1, 512], f32r)
            nc.vector.tensor_copy(ones_row[:], onesf[:])
            ones8f = const_pool.tile([8, 1], f32)
            nc.vector.memset(ones8f[:], 1.0)
            ones8 = const_pool.tile([8, 1], f32r)
            nc.vector.tensor_copy(ones8[:], ones8f[:])

            inter_init = inter_pool.tile([128, IT, CAP], bf16, name="inter")

            def emit_body():
                inter = inter_init

                # -- router stage 1 + layer-1 it=0..2, interleaved per k --
                # The PE sequencer is in-order, so the emit order must give
                # it ready work while the X^T chunks stream in: after chunk
                # k lands, the router chain and three layer-1 chains each
                # advance one k-step (~1.7 us of PE work per 0.9 us chunk).
                NPRE = 3
                lt = ps.tile([128, CAP], f32, tag="big", name="lt")
                wi_ts = {}

                def load_wi(it):
                    wi_ts[it] = wi_pool.tile(
                        [128, KT, 128], bf16, name=f"wi_{it}", tag="wi"
                    )
                    nc.sync.dma_start(wi_ts[it][:], wi_d.ap()[it])

                pre_p1 = {}
                for it in range(NPRE):
                    load_wi(it)
                    pre_p1[it] = ps.tile([128, CAP], f32, tag="big", name="p1")
                for k in range(KT):
                    for lo, hi in HALVES:
                        nc.tensor.matmul(
                            lt[0:E, lo:hi],
                            xt_sb[:, k, CAP : CAP + E],
                            xt_sb[:, k, lo:hi],
                            start=(k == 0),
                            stop=(k == KT - 1),
                        )
                    for it in range(NPRE):
                        for lo, hi in HALVES:
                            nc.tensor.matmul(
                                pre_p1[it][:, lo:hi],
                                wi_ts[it][:, k, :],
                                xt_sb[:, k, lo:hi],
                                start=(k == 0),
                                stop=(k == KT - 1),
                            )
                # exp on ACT, bias folded in (max-subtraction skipped:
                # |logits| <~ 8 so exp stays in f32 range comfortably)
                ex_sb = small_pool.tile([8, CAP], f32r, name="ex")
                for lo, hi in HALVES:
                    nc.scalar.activation(
                        ex_sb[:, lo:hi], lt[0:E, lo:hi], AF.Exp, bias=b_sb[:]
                    )
                for it in range(NPRE):
                    nc.scalar.activation(
                        inter[:, it, :], pre_p1[it][:], AF.Relu
                    )

                def emit_router_sum():
                    # -- router stage 2: sum over experts + reciprocal --
                    sm = ps.tile([128, CAP], f32, tag="big", name="sm")
                    for lo, hi in HALVES:
                        nc.tensor.matmul(
                            sm[0:1, lo:hi], ones8[:], ex_sb[:, lo:hi]
                        )
                    rc = small_pool.tile([1, CAP], f32, name="rc")
                    nc.vector.reciprocal(rc[:], sm[0:1, :])
                    w_row = small_pool.tile([1, CAP], f32r, name="w_row")
                    nc.vector.tensor_mul(w_row[:], ex_sb[0:1, :], rc[:])
                    return w_row

                def emit_router_bcast(w_row):
                    # -- router stage 3: broadcast w to 128 partitions --
                    wb = ps.tile([128, CAP], f32, tag="big", name="wb")
                    for lo, hi in HALVES:
                        nc.tensor.matmul(
                            wb[:, lo:hi], ones_row[:, 0:128], w_row[:, lo:hi]
                        )
                    wb_sb = const_pool.tile([128, CAP], f32, name="wb_sb")
                    nc.vector.tensor_copy(wb_sb[:], wb[:])
                    return wb_sb

                # all 8 layer-2 weight slabs prefetch on the scalar queue,
                # whose FIFO keeps them BEHIND the startup X^T chunks (the
                # scheduler would hoist them on the sync queue, stealing
                # serialized DMA-transfer slots from the startup path)
                wo_tiles = {}

                def prefetch_wo(ht):
                    wo_tiles[ht] = wo_pool.tile(
                        [128, IT, 128], bf16, name=f"wo_{ht}", tag="wo"
                    )
                    nc.scalar.dma_start(wo_tiles[ht][:], wo_d.ap()[ht])

                # -- layer 1 (router stages 2/3 + wo prefetch interleaved) --
                w_row = None
                wb_sb = None
                for it in range(NPRE, IT):
                    load_wi(it)
                    wi_t = wi_ts.pop(it)
                    p1 = ps.tile([128, CAP], f32, tag="big", name="p1")
                    for k in range(KT):
                        for lo, hi in HALVES:
                            nc.tensor.matmul(
                                p1[:, lo:hi],
                                wi_t[:, k, :],
                                xt_sb[:, k, lo:hi],
                                start=(k == 0),
                                stop=(k == KT - 1),
                            )
                    nc.scalar.activation(inter[:, it, :], p1[:], AF.Relu)
                    if it == NPRE + 1:
                        w_row = emit_router_sum()
                    elif it == NPRE + 3:
                        wb_sb = emit_router_bcast(w_row)
                    if it >= 8 and it <= 29 and (it - 8) % 3 == 0:
                        prefetch_wo((it - 8) // 3)

                # -- layer 2: outT = Wo^T inter^T, scale fused in copy --
                for ht in range(HT):
                    wo_t = wo_tiles.pop(ht)
                    p2 = ps.tile([128, CAP], f32, tag="big", name="p2")
                    if ht < HT - 1:
                        # halves inner: each ldweights feeds two matmuls
                        for it2 in range(IT):
                            for lo, hi in HALVES:
                                nc.tensor.matmul(
                                    p2[:, lo:hi],
                                    wo_t[:, it2, :],
                                    inter[:, it2, lo:hi],
                                    start=(it2 == 0),
                                    stop=(it2 == IT - 1),
                                )
                        o = outs_pool.tile([128, CAP], f32, name="o")
                        nc.vector.tensor_mul(o[:], p2[:], wb_sb[:])
                        nc.sync.dma_start(
                            outT_d.ap()[ht * 128 : (ht + 1) * 128, :], o[:]
                        )
                    else:
                        # last tile: column halves (PSUM-bank aligned),
                        # outer, so the scale + DMA of the first half
                        # overlaps the second half's matmul chain
                        for lo, hi in HALVES:
                            for it2 in range(IT):
                                nc.tensor.matmul(
                                    p2[:, lo:hi],
                                    wo_t[:, it2, :],
                                    inter[:, it2, lo:hi],
                                    start=(it2 == 0),
                                    stop=(it2 == IT - 1),
                                )
                            o = outs_pool.tile([128, 512], f32, name="o")
                            nc.vector.tensor_mul(
                                o[:], p2[:, lo:hi], wb_sb[:, lo:hi]
                            )
                            nc.sync.dma_start(
                                outT_d.ap()[ht * 128 : (ht + 1) * 128, lo:hi],
                                o[:],
                            )

            for _rep in range(reps):
                emit_body()

    nc.compile()
    return nc


def get_nc():
    if "nc" not in _CACHE:
        _CACHE["nc"] = _build()
    return _CACHE["nc"]


def make_in_maps(x, router_w, router_b, experts_inter, experts_out):
    import ml_dtypes

    bf = ml_dtypes.bfloat16
    x_flat = np.asarray(x, dtype=np.float32).reshape(-1, H)
    xt = np.ascontiguousarray(x_flat[:CAP].T)  # [H, CAP]
    # pack to [128, KT, CAP]: xt_p[p, k, n] = xt[k*128 + p, n]
    xt_p = np.ascontiguousarray(
        xt.reshape(KT, 128, CAP).transpose(1, 0, 2)
    ).astype(bf)

    wi_bf = np.asarray(experts_inter, dtype=np.float32).astype(bf)  # [E, H, I]
    wo_bf = np.asarray(experts_out, dtype=np.float32).astype(bf)    # [E, I, H]

    in_maps = []
    for e in range(N_CORES):
        perm = [e] + [j for j in range(E) if j != e]
        rw = np.asarray(router_w, dtype=np.float32)[perm]  # [E, H]
        rb = np.asarray(router_b, dtype=np.float32)[perm]  # [E]

        # xtr[p, k, 0:CAP] = X^T chunk; xtr[p, k, CAP+e] = rw[e, k*128+p]
        xtr = np.empty((128, KT, CAP + E), dtype=bf)
        xtr[:, :, :CAP] = xt_p
        xtr[:, :, CAP:] = (
            rw.T.reshape(KT, 128, E).transpose(1, 0, 2).astype(bf)
        )

        # wi_p[it, p, k, i] = wi[k*128+p, it*128+i]
        wi_p = np.ascontiguousarray(
            wi_bf[e].reshape(KT, 128, IT, 128).transpose(2, 1, 0, 3)
        )
        # wo_p[ht, p, it, h] = wo[it*128+p, ht*128+h]
        wo_p = np.ascontiguousarray(
            wo_bf[e].reshape(IT, 128, HT, 128).transpose(2, 1, 0, 3)
        )
        in_maps.append({
            "xt": xtr,
            "rb": np.ascontiguousarray(rb[:, None]),
            "wi": wi_p,
            "wo": wo_p,
        })
    return in_maps


def combine(results):
    partial = np.zeros((H, CAP), dtype=np.float32)
    for r in results:
        partial += r["outT"]
    out = np.zeros((B * S, H), dtype=np.float32)
    out[:CAP] = partial.T
    return out.reshape(B, S, H)


def kernel(x, router_w, router_b, experts_inter, experts_out):
    from concourse import bass_utils

    nc = get_nc()
    in_maps = make_in_maps(x, router_w, router_b, experts_inter, experts_out)
    res = bass_utils.run_bass_kernel_spmd(nc, in_maps, core_ids=list(range(N_CORES)))
    return combine(res.results)


# revision 17
# speedup vs baseline: 4.9675x; 1.0032x over previous
"""MoE routing kernel for Trainium2 (8 NeuronCores, expert-parallel).

Problem (hardcoded shapes): B=4, S=2048, H=1024, I=4096, E=8, capacity=1024.

Mathematical simplification of the reference: softmax routing weights are
strictly positive, so the routing mask is all-ones and the stable argsort of
the (constant) mask is the identity permutation.  Consequently every expert
processes exactly tokens 0..1023 of the flattened [8192, 1024] input, and the
output is nonzero only for those tokens:

    out[n] = sum_e softmax(x[n] @ Wr.T + b)[e] * (relu(x[n] @ Wi[e]) @ Wo[e])

Sharding: expert-parallel.  Each of the 8 cores receives the same 1024-token
slice (pre-transposed to X^T on host) and the weights of ONE expert; it
computes that expert's weighted output transposed, [1024 H, 1024 tok].  The
host sums the 8 partial outputs (the MoE combine), transposes once, and
scatters into the full [4, 2048, 1024] zero tensor.

Per-core device computation (v4, all-bf16 data path):
  router:   logits^T[E, tok] = Wr_perm X^T (PE, bf16); exp with the bias
            folded into the ACT instruction; sum over the partition dim via
            ones-matmul; the softmax stages are interleaved into early
            layer-1 iterations so the PE never stalls on the DVE/ACT chain.
  layer 1:  inter^T[I, tok] = relu(Wi^T X^T)   (bf16 matmuls, bf16 store)
  layer 2:  outT[H, tok] = Wo^T inter^T        (bf16 matmuls),
            routing-weight scale fused into the PSUM->SBUF output copy.

DMA plan: HWDGE dispatches round-robin across the two queues and serialize
at ~0.6 us each, so the startup is laid out to get X^T flowing immediately:
the router weights ride packed inside the X^T k-chunks (columns CAP..CAP+7),
chunk 0 is the sync queue's first entry, chunks 1-7 go on the scalar queue
(whose FIFO order the scheduler preserves), and the 32 Wi tiles stream on
the sync queue through a 4-deep pool.  All 8 Wo slabs prefetch on the
scalar queue during layer 1, safely behind the startup chunks.  One shared
4-slot PSUM pool (8 banks) lets the router-logit chain plus three layer-1
chains stay open while X^T streams in; the emit order advances all four
chains one k-step per arriving chunk so the in-order PE sequencer always
has ready work.  The last output tile is processed in column halves
(PSUM-bank aligned) so the final scale-multiply + DMA pipelines into the
second half's matmul chain, shortening the tail.
"""

import numpy as np

_CACHE = {}

B, S, H, I, E = 4, 2048, 1024, 4096, 8
CAP = 1024  # capacity = ceil(B*S/E)
N_CORES = 8
KT = H // 128   # 8 k-tiles (H on partitions)
IT = I // 128   # 32 I-tiles
HT = H // 128   # 8 output H-tiles

HALVES = ((0, 512), (512, 1024))


def _build(reps=1):
    import concourse.bacc as bacc
    import concourse.mybir as mybir
    import concourse.tile as tile

    f32 = mybir.dt.float32
    f32r = mybir.dt.float32r
    bf16 = mybir.dt.bfloat16
    AF = mybir.ActivationFunctionType

    nc = bacc.Bacc("TRN2", target_bir_lowering=False, debug=False)

    # X^T k-chunks with the (permuted) router weight columns for that k-tile
    # packed at columns CAP..CAP+E: the first DMA dispatch then carries both
    # the first X^T chunk and its router weights (HWDGE dispatches serialize
    # at ~0.6 us each, so every separate startup DMA delays the first matmul)
    xt_d = nc.dram_tensor("xt", [128, KT, CAP + E], bf16, kind="ExternalInput")
    rb_d = nc.dram_tensor("rb", [E, 1], f32, kind="ExternalInput")
    wi_d = nc.dram_tensor("wi", [IT, 128, KT, 128], bf16, kind="ExternalInput")
    wo_d = nc.dram_tensor("wo", [HT, 128, IT, 128], bf16, kind="ExternalInput")
    outT_d = nc.dram_tensor("outT", [H, CAP], f32, kind="ExternalOutput")

    with tile.TileContext(nc) as tc:
        with (
            tc.tile_pool(name="const", bufs=1) as const_pool,
            tc.tile_pool(name="wi", bufs=4) as wi_pool,
            tc.tile_pool(name="wo", bufs=8) as wo_pool,
            tc.tile_pool(name="inter", bufs=1) as inter_pool,
            tc.tile_pool(name="outs", bufs=2) as outs_pool,
            tc.tile_pool(name="small", bufs=2) as small_pool,
            tc.tile_pool(name="ps", bufs=4, space="PSUM") as ps,
        ):
            # ---- startup DMAs ----
            # one DMA per k-chunk (X^T + router weight columns).  Chunk 0
            # goes on the sync queue AHEAD of the wi stream so it is the
            # very first HWDGE dispatch (dispatches round-robin across the
            # two queues and serialize at ~0.6 us each); chunks 1..7 ride
            # the scalar queue.  The bias is dispatched last — it is only
            # needed once the exp fires (~9 us).
            xt_sb = const_pool.tile([128, KT, CAP + E], bf16)
            nc.sync.dma_start(xt_sb[:, 0, :], xt_d.ap()[:, 0, :])
            for k in range(1, KT):
                nc.scalar.dma_start(xt_sb[:, k, :], xt_d.ap()[:, k, :])
            b_sb = const_pool.tile([E, 1], f32)
            nc.scalar.dma_start(b_sb[:], rb_d.ap())

            onesf = const_pool.tile([1, 512], f32)
            nc.vector.memset(onesf[:], 1.0)
            ones_row = const_pool.tile([1, 512], f32r)
            nc.vector.tensor_copy(ones_row[:], onesf[:])
            ones8f = const_pool.tile([8, 1], f32)
            nc.vector.memset(ones8f[:], 1.0)
            ones8 = const_pool.tile([8, 1], f32r)
            nc.vector.tensor_copy(ones8[:], ones8f[:])

            inter_init = inter_pool.tile([128, IT, CAP], bf16, name="inter")

            def emit_body():
                inter = inter_init

                # -- router stage 1 + layer-1 it=0..2, interleaved per k --
                # The PE sequencer is in-order, so the emit order must give
                # it ready work while the X^T chunks stream in: after chunk
                # k lands, the router chain and three layer-1 chains each
                # advance one k-step (~1.7 us of PE work per 0.9 us chunk).
                NPRE = 3
                lt = ps.tile([128, CAP], f32, tag="big", name="lt")
                wi_ts = {}

                def load_wi(it):
                    wi_ts[it] = wi_pool.tile(
                        [128, KT, 128], bf16, name=f"wi_{it}", tag="wi"
                    )
                    nc.sync.dma_start(wi_ts[it][:], wi_d.ap()[it])

                pre_p1 = {}
                for it in range(NPRE):
                    load_wi(it)
                    pre_p1[it] = ps.tile([128, CAP], f32, tag="big", name="p1")
                for k in range(KT):
                    for lo, hi in HALVES:
                        nc.tensor.matmul(
                            lt[0:E, lo:hi],
                            xt_sb[:, k, CAP : CAP + E],
                            xt_sb[:, k, lo:hi],
                            start=(k == 0),
                            stop=(k == KT - 1),
                        )
                    for it in range(NPRE):
                        for lo, hi in HALVES:
                            nc.tensor.matmul(
                                pre_p1[it][:, lo:hi],
                                wi_ts[it][:, k, :],
                                xt_sb[:, k, lo:hi],
                                start=(k == 0),
                                stop=(k == KT - 1),
                            )
                # exp on ACT, bias folded in (max-subtraction skipped:
                # |logits| <~ 8 so exp stays in f32 range comfortably)
                ex_sb = small_pool.tile([8, CAP], f32r, name="ex")
                for lo, hi in HALVES:
                    nc.scalar.activation(
                        ex_sb[:, lo:hi], lt[0:E, lo:hi], AF.Exp, bias=b_sb[:]
                    )
                for it in range(NPRE):
                    nc.scalar.activation(
                        inter[:, it, :], pre_p1[it][:], AF.Relu
                    )

                def emit_router_sum():
                    # -- router stage 2: sum over experts + reciprocal --
                    sm = ps.tile([128, CAP], f32, tag="big", name="sm")
                    for lo, hi in HALVES:
                        nc.tensor.matmul(
                            sm[0:1, lo:hi], ones8[:], ex_sb[:, lo:hi]
                        )
                    rc = small_pool.tile([1, CAP], f32, name="rc")
                    nc.vector.reciprocal(rc[:], sm[0:1, :])
                    w_row = small_pool.tile([1, CAP], f32r, name="w_row")
                    nc.vector.tensor_mul(w_row[:], ex_sb[0:1, :], rc[:])
                    return w_row

                def emit_router_bcast(w_row):
                    # -- router stage 3: broadcast w to 128 partitions --
                    wb = ps.tile([128, CAP], f32, tag="big", name="wb")
                    for lo, hi in HALVES:
                        nc.tensor.matmul(
                            wb[:, lo:hi], ones_row[:, 0:128], w_row[:, lo:hi]
                        )
                    wb_sb = const_pool.tile([128, CAP], f32, name="wb_sb")
                    nc.vector.tensor_copy(wb_sb[:], wb[:])
                    return wb_sb

                # all 8 layer-2 weight slabs prefetch on the scalar queue,
                # whose FIFO keeps them BEHIND the startup X^T chunks (the
                # scheduler would hoist them on the sync queue, stealing
                # serialized DMA-transfer slots from the startup path)
                wo_tiles = {}

                def prefetch_wo(ht):
                    wo_tiles[ht] = wo_pool.tile(
                        [128, IT, 128], bf16, name=f"wo_{ht}", tag="wo"
                    )
                    nc.scalar.dma_start(wo_tiles[ht][:], wo_d.ap()[ht])

                # -- layer 1 (router stages 2/3 + wo prefetch interleaved) --
                w_row = None
                wb_sb = None
                for it in range(NPRE, IT):
                    load_wi(it)
                    wi_t = wi_ts.pop(it)
                    p1 = ps.tile([128, CAP], f32, tag="big", name="p1")
                    for k in range(KT):
                        for lo, hi in HALVES:
                            nc.tensor.matmul(
                                p1[:, lo:hi],
                                wi_t[:, k, :],
                                xt_sb[:, k, lo:hi],
                                start=(k == 0),
                                stop=(k == KT - 1),
                            )
                    nc.scalar.activation(inter[:, it, :], p1[:], AF.Relu)
                    if it == NPRE + 1:
                        w_row = emit_router_sum()
                    elif it == NPRE + 3:
                        wb_sb = emit_router_bcast(w_row)
                    if it >= 8 and it <= 29 and (it - 8) % 3 == 0:
                        prefetch_wo((it - 8) // 3)

                # -- layer 2: outT = Wo^T inter^T, scale fused in copy --
                for ht in range(HT):
                    wo_t = wo_tiles.pop(ht)
                    if ht < HT - 1:
                        # halves inner: each ldweights feeds two matmuls
                        p2 = ps.tile([128, CAP], f32, tag="big", name="p2")
                        for it2 in range(IT):
                            for lo, hi in HALVES:
                                nc.tensor.matmul(
                                    p2[:, lo:hi],
                                    wo_t[:, it2, :],
                                    inter[:, it2, lo:hi],
                                    start=(it2 == 0),
                                    stop=(it2 == IT - 1),
                                )
                        o = outs_pool.tile([128, CAP], f32, name="o")
                        nc.vector.tensor_mul(o[:], p2[:], wb_sb[:])
                        nc.sync.dma_start(
                            outT_d.ap()[ht * 128 : (ht + 1) * 128, :], o[:]
                        )
                    else:
                        # last tile: column halves, outer, in SEPARATE PSUM
                        # tiles (a shared tile makes half 2's start=True
                        # zeroing wait on the DVE still reading half 1), so
                        # the scale + DMA of the first half overlaps the
                        # second half's matmul chain.  The final half's
                        # scale-multiply + DMA go out in 256-col pieces so
                        # the last DMA starts as early as possible.
                        for lo, hi in HALVES:
                            p2 = ps.tile(
                                [128, 512], f32, tag="big", name="p2l"
                            )
                            for it2 in range(IT):
                                nc.tensor.matmul(
                                    p2[:],
                                    wo_t[:, it2, :],
                                    inter[:, it2, lo:hi],
                                    start=(it2 == 0),
                                    stop=(it2 == IT - 1),
                                )
                            if lo == 0:
                                o = outs_pool.tile([128, 512], f32, name="o")
                                nc.vector.tensor_mul(
                                    o[:], p2[:], wb_sb[:, lo:hi]
                                )
                                nc.sync.dma_start(
                                    outT_d.ap()[
                                        ht * 128 : (ht + 1) * 128, lo:hi
                                    ],
                                    o[:],
                                )
                            else:
                                for q in range(2):
                                    qlo = lo + q * 256
                                    o = outs_pool.tile(
                                        [128, 256], f32, name="o"
                                    )
                                    nc.vector.tensor_mul(
                                        o[:],
                                        p2[:, q * 256 : (q + 1) * 256],
                                        wb_sb[:, qlo : qlo + 256],
                                    )
                                    nc.sync.dma_start(
                                        outT_d.ap()[
                                            ht * 128 : (ht + 1) * 128,
                                            qlo : qlo + 256,
                                        ],
                                        o[:],
                                    )

            for _rep in range(reps):
                emit_body()

    nc.compile()
    return nc


def get_nc():
    if "nc" not in _CACHE:
        _CACHE["nc"] = _build()
    return _CACHE["nc"]


def make_in_maps(x, router_w, router_b, experts_inter, experts_out):
    import ml_dtypes

    bf = ml_dtypes.bfloat16
    x_flat = np.asarray(x, dtype=np.float32).reshape(-1, H)
    xt = np.ascontiguousarray(x_flat[:CAP].T)  # [H, CAP]
    # pack to [128, KT, CAP]: xt_p[p, k, n] = xt[k*128 + p, n]
    xt_p = np.ascontiguousarray(
        xt.reshape(KT, 128, CAP).transpose(1, 0, 2)
    ).astype(bf)

    wi_bf = np.asarray(experts_inter, dtype=np.float32).astype(bf)  # [E, H, I]
    wo_bf = np.asarray(experts_out, dtype=np.float32).astype(bf)    # [E, I, H]

    in_maps = []
    for e in range(N_CORES):
        perm = [e] + [j for j in range(E) if j != e]
        rw = np.asarray(router_w, dtype=np.float32)[perm]  # [E, H]
        rb = np.asarray(router_b, dtype=np.float32)[perm]  # [E]

        # xtr[p, k, 0:CAP] = X^T chunk; xtr[p, k, CAP+e] = rw[e, k*128+p]
        xtr = np.empty((128, KT, CAP + E), dtype=bf)
        xtr[:, :, :CAP] = xt_p
        xtr[:, :, CAP:] = (
            rw.T.reshape(KT, 128, E).transpose(1, 0, 2).astype(bf)
        )

        # wi_p[it, p, k, i] = wi[k*128+p, it*128+i]
        wi_p = np.ascontiguousarray(
            wi_bf[e].reshape(KT, 128, IT, 128).transpose(2, 1, 0, 3)
        )
        # wo_p[ht, p, it, h] = wo[it*128+p, ht*128+h]
        wo_p = np.ascontiguousarray(
            wo_bf[e].reshape(IT, 128, HT, 128).transpose(2, 1, 0, 3)
        )
        in_maps.append({
            "xt": xtr,
            "rb": np.ascontiguousarray(rb[:, None]),
            "wi": wi_p,
            "wo": wo_p,
        })
    return in_maps


def combine(results):
    partial = np.zeros((H, CAP), dtype=np.float32)
    for r in results:
        partial += r["outT"]
    out = np.zeros((B * S, H), dtype=np.float32)
    out[:CAP] = partial.T
    return out.reshape(B, S, H)


def kernel(x, router_w, router_b, experts_inter, experts_out):
    from concourse import bass_utils

    nc = get_nc()
    in_maps = make_in_maps(x, router_w, router_b, experts_inter, experts_out)
    res = bass_utils.run_bass_kernel_spmd(nc, in_maps, core_ids=list(range(N_CORES)))
    return combine(res.results)
